# revision 21
# baseline (speedup 1.0000x reference)
"""Bass/Tile TRN2 kernel for nn_BiDirectionalAttention (8-core SPMD), v3.

Math (reference):
    qc[c,q]   = sum_d H[c,d]*w_qc[d]*U[q,d] + b_qc
    s         = qc + (U@w_q + b_q)[None,:] + (H@w_c + b_c)[:,None]
    A         = softmax(s, axis=0)            # over context dim c (sharded)
    U_toggler = A @ U                          # [c_len, D]
    b         = max(H, axis=1); c2q = softmax(b)
    H_toggler = broadcast(c2q @ H)             # every row identical

Exact simplifications:
  * b_q/b_c/b_qc and q_term are constant along the softmax axis -> cancel.
  * c_term folds into gemm1: s^T[q,c] = sum_d (U^T[d,q]*w_qc[d] + w_c[d]) * H^T[d,c]
  * |s| <= ~12 -> softmax without max-subtraction is exact in fp32.
  * 1/S_glob[q] folds into U (per-row scale) instead of into E.
  * H_toggler cross-core reduction happens on the HOST: each core outputs its
    local row partial (e_b @ H_shard) and e_b partition sums.

v4 collective restructure (v2/v3 spent ~40us of PE idle around the AG):
  * A cold ncfw mesh takes ~11us to begin plus first-sync skew; a warm one
    begins in <1us. So: a warmup AG whose input is a host-fed zeros
    ExternalInput posts its trigger before any input-load descriptors and
    completes during gemm1; the single real stats AG (4 KiB) then runs on a
    warm, synced mesh right at gemm1's end.
  * Collective triggers and sem-gated DMA descriptors share in-order rings
    with everything else (head-of-line blocking!), so nothing sem-gated is
    posted between the input loads and the AG-output (agg) descriptor.
  * All inputs bf16: 8 MiB/core total DMA, rings quiet well before the AG.
  * H_toggler GEMV PSUM evacuation on the scalar engine so the vector queue
    cannot stall it behind the AG-dependent reduction ops.
"""

import numpy as np
import ml_dtypes

import concourse.bass as bass
import concourse.mybir as mybir
import concourse.tile as tile
from concourse import bacc
from concourse.bass_utils import run_bass_kernel_spmd

P = 128
N_CORES = 8
C_LEN, Q_LEN, D = 8192, 1024, 1024

F32 = mybir.dt.float32
F32R = mybir.dt.float32r
BF16 = mybir.dt.bfloat16
AX = mybir.AxisListType.X
ALU = mybir.AluOpType
ACTF = mybir.ActivationFunctionType
NCH = 512  # matmul moving-operand chunk
N_WARM = 20  # PE p-state warm-up matmuls


def build_nc(c_sh=C_LEN // N_CORES, q_len=Q_LEN, d=D, n_cores=N_CORES):
    assert c_sh % NCH == 0 and q_len % P == 0 and d % NCH == 0
    CT, QT, DT = c_sh // P, q_len // P, d // P
    NC_C = c_sh // NCH  # c chunks (gemm1 moving dim)
    NC_D = d // NCH  # d chunks (gemm2 moving dim)
    QT2 = QT // 2  # q-tiles per AG half

    nc = bacc.Bacc(
        "TRN2", target_bir_lowering=False, debug=False, num_devices=n_cores
    )
    ht_d = nc.dram_tensor("ht", [d, c_sh], BF16, kind="ExternalInput")
    ut_d = nc.dram_tensor("ut", [d, q_len], BF16, kind="ExternalInput")
    u_d = nc.dram_tensor("u", [q_len, d], BF16, kind="ExternalInput")
    h_d = nc.dram_tensor("h", [c_sh, d], BF16, kind="ExternalInput")
    # host-prearranged [P, DT] with w[dt*128+p] at [p, dt]
    w_qc = nc.dram_tensor("w_qc_t", [P, DT], F32, kind="ExternalInput")
    w_c = nc.dram_tensor("w_c_t", [P, DT], F32, kind="ExternalInput")
    out_ut = nc.dram_tensor("out_ut", [c_sh, d], F32, kind="ExternalOutput")
    # local H_toggler stats: row partial [d] + e_b partition sums [P]
    out_st = nc.dram_tensor("out_st", [d + P], F32, kind="ExternalOutput")

    # pre-tiled DRAM views: [p, tile, inner]
    ht_v = ht_d.rearrange("(t p) c -> p t c", p=P)
    ut_v = ut_d.rearrange("(t p) q -> p t q", p=P)
    u_v = u_d.rearrange("(t p) d -> p t d", p=P)
    h_v = h_d.rearrange("(t p) d -> p t d", p=P)

    ST2 = P * QT2

    with tile.TileContext(nc) as tc:
        with (
            tc.tile_pool(name="persist", bufs=1) as persist,
            tc.tile_pool(name="outp", bufs=3) as outp,
            tc.tile_pool(name="dram", bufs=1, space="DRAM") as dram,
            tc.tile_pool(name="pp_mm", bufs=5, space="PSUM") as pp_mm,
            tc.tile_pool(name="pp_dmy", bufs=1, space="PSUM") as pp_dmy,
            tc.tile_pool(name="pp_row", bufs=2, space="PSUM") as pp_row,
        ):
            cc_in = dram.tile([P * QT], F32, name="cc_in", tag="cc_in")
            cc_ag = dram.tile(
                [n_cores * P * QT], F32, name="cc_ag", tag="cc_ag",
                addr_space="Shared",
            )
            wu_out = dram.tile(
                [n_cores * P], F32, name="wu_out", tag="wu_out",
                addr_space="Shared",
            )
            # warmup AG posted before all input loads: the cold-mesh setup
            # (~11us begin lag + first-sync skew, and it can't start until
            # the DMA rings quiesce) happens while gemm1 runs; the real AG
            # then runs on a warm, synced mesh.
            wu_in = dram.tile([P], F32, name="wu_in", tag="wu_in")
            wu_z = persist.tile([1, P], F32, name="wu_z", tag="wu_z")
            nc.gpsimd.memset(wu_z, 0.0)
            nc.sync.dma_start(wu_in[:], wu_z)
            nc.gpsimd.collective_compute(
                "AllGather",
                ALU.bypass,
                replica_groups=[list(range(n_cores))],
                ins=[wu_in[:]],
                outs=[wu_out[:]],
            )

            # ---- tiny constants ----
            wqc_sb = persist.tile([P, DT], F32, name="wqc_sb", tag="wqc_sb")
            wc_sb = persist.tile([P, DT], F32, name="wc_sb", tag="wc_sb")
            nc.sync.dma_start(wqc_sb, w_qc[:, :])
            nc.sync.dma_start(wc_sb, w_c[:, :])

            # ---- PE p-state warm-up: dummy matmuls on memset tiles ----
            dmy_w = persist.tile([P, P], BF16, name="dmy_w", tag="dmy_w")
            dmy_x = persist.tile([P, NCH], BF16, name="dmy_x", tag="dmy_x")
            nc.gpsimd.memset(dmy_w, 0.0)
            nc.gpsimd.memset(dmy_x, 0.0)
            ps_dmy = pp_dmy.tile([P, NCH], F32, name="ps_dmy", tag="ps_dmy")
            for i in range(N_WARM):
                nc.tensor.matmul(
                    ps_dmy, lhsT=dmy_w, rhs=dmy_x,
                    start=(i == 0), stop=(i == N_WARM - 1),
                )

            # ---- gemm1 operands, in need-order ----
            # lhsT1[p, dt, q] = U^T*w_qc + w_c (bf16 DMA, then in-place DVE)
            # hT[p, dt, c]    = H^T bf16
            lhsT1 = persist.tile([P, DT, q_len], BF16, name="lhsT1", tag="lhsT1")
            hT_t = [
                persist.tile([P, DT, NCH], BF16, name=f"hT{j}", tag=f"hT{j}")
                for j in range(NC_C)
            ]

            QCH0 = min(P, q_len)
            q_chunks = [(0, QCH0)] + (
                [(QCH0, q_len - QCH0)] if q_len > QCH0 else []
            )

            def load_lhsT1(off, ln):
                nc.sync.dma_start(
                    lhsT1[:, :, off : off + ln], ut_v[:, :, off : off + ln]
                )
                for dt in range(DT):
                    nc.vector.tensor_scalar(
                        out=lhsT1[:, dt, off : off + ln],
                        in0=lhsT1[:, dt, off : off + ln],
                        scalar1=wqc_sb[:, dt : dt + 1],
                        scalar2=wc_sb[:, dt : dt + 1],
                        op0=ALU.mult,
                        op1=ALU.add,
                    )

            load_lhsT1(*q_chunks[0])
            for j in range(NC_C):
                nc.sync.dma_start(
                    hT_t[j], ht_v[:, :, j * NCH : (j + 1) * NCH]
                )
            for ch in q_chunks[1:]:
                load_lhsT1(*ch)

            # ---- natural-layout H (bf16): rowmax + GEMV ----
            h_nat = persist.tile([P, CT, d], BF16, name="h_nat", tag="h_nat")
            HC = max(CT // 2, 1)
            for t0 in range(0, CT, HC):
                nc.sync.dma_start(
                    h_nat[:, t0 : t0 + HC, :], h_v[:, t0 : t0 + HC, :]
                )
            # ---- natural-layout U (gemm2 rhs) ----
            u_sb = persist.tile([P, QT, d], BF16, name="u_sb", tag="u_sb")
            nc.sync.dma_start(u_sb, u_v[:, :, :])

            # rowmax (free-axis reduce is vector-only; overlaps gemm1)
            b_loc = persist.tile([P, CT], F32, name="b_loc", tag="b_loc")
            for ct in range(CT):
                nc.vector.reduce_max(
                    out=b_loc[:, ct : ct + 1], in_=h_nat[:, ct, :], axis=AX
                )

            # ---- gemm1: s^T = lhsT1^T @ H^T ; E = exp(s^T) bf16; S_local ----
            e_sb = [
                persist.tile([P, c_sh], BF16, name=f"e_sb{mt}", tag=f"e_sb{mt}")
                for mt in range(QT)
            ]
            s_part = persist.tile([P, QT, NC_C], F32, name="s_part", tag="s_part")
            stats = persist.tile([P, QT], F32, name="stats", tag="stats")
            for mt in range(QT):
                for j in range(NC_C):
                    ps = pp_mm.tile([P, NCH], F32, name="ps_mm", tag="ps_mm")
                    for kt in range(DT):
                        nc.tensor.matmul(
                            ps,
                            lhsT=lhsT1[:, kt, mt * P : (mt + 1) * P],
                            rhs=hT_t[j][:, kt, :],
                            start=(kt == 0),
                            stop=(kt == DT - 1),
                        )
                    nc.scalar.activation(
                        out=e_sb[mt][:, j * NCH : (j + 1) * NCH],
                        in_=ps,
                        func=ACTF.Exp,
                        accum_out=s_part[:, mt, j : j + 1],
                    )
                nc.vector.reduce_sum(
                    out=stats[:, mt : mt + 1], in_=s_part[:, mt, :], axis=AX
                )
            # single stats AG (4 KiB) on a warm mesh
            nc.sync.dma_start(cc_in.rearrange("(p o) -> p o", p=P), stats)
            nc.gpsimd.collective_compute(
                "AllGather",
                ALU.bypass,
                replica_groups=[list(range(n_cores))],
                ins=[cc_in[:]],
                outs=[cc_ag[:]],
            )

            # ---- H_toggler GEMV on PE (overlaps the AllGather wait) ----
            # e_b = exp(b); row[d] = sum_ct e_b(:,ct)^T @ H[:,ct,:]
            e_b = persist.tile([P, CT], BF16, name="e_b", tag="e_b")
            nc.scalar.activation(e_b, b_loc, ACTF.Exp)
            st_row = persist.tile([1, d], F32, name="st_row", tag="st_row")
            for j in range(NC_D):
                ps_r = pp_row.tile([1, NCH], F32, name="ps_row", tag="ps_row")
                for ct in range(CT):
                    nc.tensor.matmul(
                        ps_r,
                        lhsT=e_b[:, ct : ct + 1],
                        rhs=h_nat[:, ct, j * NCH : (j + 1) * NCH],
                        start=(ct == 0),
                        stop=(ct == CT - 1),
                    )
                # scalar engine so the vector queue (AG-dependent ops)
                # can't stall it; Pool has no PSUM access
                nc.scalar.activation(
                    out=st_row[:, j * NCH : (j + 1) * NCH],
                    in_=ps_r,
                    func=ACTF.Copy,
                )
            st_ebs = persist.tile([P, 1], F32, name="st_ebs", tag="st_ebs")
            nc.vector.reduce_sum(out=st_ebs, in_=e_b[:, :], axis=AX)
            # keep the PE clock hot through the AG wait (idle >3us drops the
            # p-state; gemm2's first matmuls ran 1.6-2.3x slow in v4b)
            ps_dmy2 = pp_dmy.tile([P, NCH], F32, name="ps_dmy2", tag="ps_dmy")
            for i in range(56):
                nc.tensor.matmul(
                    ps_dmy2, lhsT=dmy_w, rhs=dmy_x,
                    start=(i == 0), stop=(i == 55),
                )

            # ---- AG result -> S_glob -> fold 1/S into U rows ----
            # (agg posted before out_st so no sem-gated descriptor sits
            # ahead of the AG-output path in the DMA rings)
            agg = persist.tile([P, n_cores, QT], F32, name="agg", tag="agg")
            nc.sync.dma_start(
                agg, cc_ag.rearrange("(r p o) -> p r o", p=P, o=QT)
            )
            nc.sync.dma_start(
                out_st[0:d].rearrange("(p o) -> p o", p=1), st_row
            )
            nc.sync.dma_start(
                out_st[d : d + P].rearrange("(p o) -> p o", p=P), st_ebs
            )
            stats2 = persist.tile([P, QT], F32, name="stats2", tag="stats2")
            gh = persist.tile([P, QT], F32, name="gh", tag="gh")
            gh2 = persist.tile([P, QT], F32, name="gh2", tag="gh2")
            dh = persist.tile([P, QT], F32, name="dh", tag="dh")
            # tree reduce split across gpsimd (slots 4..7) and vector
            # (slots 0..3) so the critical DVE path is 3 adds, not 7
            nc.gpsimd.tensor_add(out=gh, in0=agg[:, 4, :], in1=agg[:, 5, :])
            nc.gpsimd.tensor_add(out=gh2, in0=agg[:, 6, :], in1=agg[:, 7, :])
            nc.gpsimd.tensor_add(out=gh, in0=gh, in1=gh2)
            nc.vector.tensor_add(out=dh, in0=agg[:, 0, :], in1=agg[:, 1, :])
            nc.vector.tensor_add(out=stats2, in0=agg[:, 2, :], in1=agg[:, 3, :])
            nc.vector.tensor_add(out=dh, in0=dh, in1=stats2)
            nc.vector.tensor_add(out=stats2, in0=dh, in1=gh)
            rs_all = persist.tile([P, QT], F32, name="rs_all", tag="rs_all")
            nc.vector.reciprocal(rs_all, stats2)
            for qt in range(QT):
                nc.vector.tensor_scalar_mul(
                    u_sb[:, qt, :], u_sb[:, qt, :], rs_all[:, qt : qt + 1]
                )

            # ---- gemm2: U_toggler[c,:] = E^T-slices @ (U/S) ----
            for mt in range(CT):
                ot = outp.tile([P, d], F32, name="ot", tag="ot")
                for j in range(NC_D):
                    ps = pp_mm.tile([P, NCH], F32, name="ps_mm", tag="ps_mm")
                    for kt in range(QT):
                        nc.tensor.matmul(
                            ps,
                            lhsT=e_sb[kt][:, mt * P : (mt + 1) * P],
                            rhs=u_sb[:, kt, j * NCH : (j + 1) * NCH],
                            start=(kt == 0),
                            stop=(kt == QT - 1),
                        )
                    if j % 2 == 0:
                        nc.vector.tensor_copy(
                            out=ot[:, j * NCH : (j + 1) * NCH], in_=ps
                        )
                    else:
                        nc.scalar.activation(
                            out=ot[:, j * NCH : (j + 1) * NCH],
                            in_=ps,
                            func=ACTF.Copy,
                        )
                    nc.sync.dma_start(
                        out_ut[mt * P : (mt + 1) * P, j * NCH : (j + 1) * NCH],
                        ot[:, j * NCH : (j + 1) * NCH],
                    )

    nc.finalize()
    return nc


_CACHE = {}


def _get_nc():
    if "nc" not in _CACHE:
        _CACHE["nc"] = build_nc()
    return _CACHE["nc"]


def make_in_maps(H, U, w_qc, w_c, n_cores=N_CORES):
    c_sh = H.shape[0] // n_cores
    d = H.shape[1]
    BF = ml_dtypes.bfloat16
    H_bf = np.ascontiguousarray(H.astype(BF))
    HT = np.ascontiguousarray(H_bf.T)
    UT = np.ascontiguousarray(U.T.astype(BF))
    U_bf = np.ascontiguousarray(U.astype(BF))
    wqc_t = np.ascontiguousarray(w_qc.reshape(d // P, P).T)
    wc_t = np.ascontiguousarray(w_c.reshape(d // P, P).T)
    return [
        {
            "h": np.ascontiguousarray(H_bf[i * c_sh : (i + 1) * c_sh]),
            "ht": np.ascontiguousarray(HT[:, i * c_sh : (i + 1) * c_sh]),
            "u": U_bf,
            "ut": UT,
            "w_qc_t": wqc_t,
            "w_c_t": wc_t,
        }
        for i in range(n_cores)
    ]


def decode_row(st_list, d=D):
    """st_list: per-core out_st [d+P] -> H_toggler row [d] (host reduce)."""
    row = np.zeros(d, np.float64)
    bsum = 0.0
    for st in st_list:
        st = np.asarray(st, np.float64).reshape(-1)
        row += st[0:d]
        bsum += st[d:].sum()
    return (row / bsum).astype(np.float32)


def _run(H, U, w_qc, w_c, trace=False):
    in_maps = make_in_maps(H, U, w_qc, w_c)
    return run_bass_kernel_spmd(
        _get_nc(), in_maps, list(range(N_CORES)), trace=trace
    )


def kernel(H, U, w_q, b_q, w_c, b_c, w_qc, b_qc):
    # w_q/b_q/b_c/b_qc shift softmax logits by a per-column constant and
    # cancel exactly; they are unused.
    H = np.ascontiguousarray(np.asarray(H, dtype=np.float32))
    U = np.ascontiguousarray(np.asarray(U, dtype=np.float32))
    w_c = np.ascontiguousarray(np.asarray(w_c, dtype=np.float32))
    w_qc = np.ascontiguousarray(np.asarray(w_qc, dtype=np.float32))
    res = _run(H, U, w_qc, w_c).results
    U_toggler = np.concatenate([r["out_ut"] for r in res], axis=0)
    row = decode_row([r["out_st"] for r in res])
    H_toggler = np.broadcast_to(row, H.shape).copy()
    return (U_toggler, H_toggler)
